# revision 26
# baseline (speedup 1.0000x reference)
"""MoE block (AdaptFormer adapters, top-2 of 8 experts) on 8 TRN2 NeuronCores.

Data-parallel over the 8192 tokens (1024/core), router + expert adapter
weights replicated. bf16 GEMMs, exact hi/lo router logits.

Per core:
  - x ships as an exact bf16 hi/lo split (xh + xl == x to ~2^-17),
    pre-transposed AND block-major on the host: [NLB*128, KC*512] where
    slab b holds block b's 512 tokens for all 8 D-chunks. Pure layout
    prep; lets block-0 compute start as soon as ~0.5MB has landed.
  - logits: one PSUM group of 16 matmuls per block computes
    (xh+xl) @ (wgh|wgl) with the packed [wgh|wgl] stationary: rows 0:8
    collect xh@wgh + xl@wgh, rows 8:16 collect xh@wgl + xl@wgl; the
    per-token transpose then a small add folds the halves. Exact to
    ~5e-6 (top-2/3 logit gaps below that are coin-flips worth <1e-2
    rel_l2 on this dataset).
  - experts: HT = Wd^T-stationary bf16 matmuls on xh chunks (c-outer,
    4 PSUM banks in flight so compute paces the wd stream) -> relu ->
    bf16; per-token gates broadcast across each expert's 64 bottleneck
    lanes by GpSimd partition_broadcast (no PE work); HG = relu * GB on
    DVE -> bf16; OUT tiles = HG-slices @ Wu_flat (bf16) accumulated
    over the expert axis.
  - gating tiles are emission-interleaved with HT/OUT matmul groups so
    the PE never idles waiting on the DVE softmax chain.
  - output is written bf16 [TC, D] and converted to f32 on host.
All experts computed densely; sparse gates zero the non-top-2 terms
(mathematically identical to dispatch/combine). The PE row budget is
kept minimal because TRN2's HAM power manager clamps sustained Tensor
utilization (~2/3 duty at full load): warm-up is 4 matmuls, the gate
expansion runs on GpSimd, and everything else is bf16 single-pass.
"""
import numpy as np
import ml_dtypes
from contextlib import ExitStack

import concourse.bass as bass
import concourse.tile as tile
from concourse import bacc, mybir
from concourse.bass_utils import run_bass_kernel_spmd

N_CORES = 8
B_DIM, S_DIM, D = 2, 4096, 1024
T = B_DIM * S_DIM          # 8192 tokens
TC = T // N_CORES          # 1024 tokens per core
E, BK = 8, 64              # experts, bottleneck
EB = E * BK                # 512 concatenated expert axis
P = 128
KC = D // P                # D chunks
BC = EB // P               # bottleneck chunks
LBLK = 512                 # token block
NLB = TC // LBLK           # 2 blocks
TPB = LBLK // P            # token tiles per block
SCALE = 0.5
N_WARM = 2                 # PE warm-up matmuls: minimal (HAM power envelope -
                           # every wasted row costs grant budget)

F32 = mybir.dt.float32
F32R = mybir.dt.float32r
BF16 = mybir.dt.bfloat16
AL = mybir.AluOpType
ACTF = mybir.ActivationFunctionType
AX = mybir.AxisListType

_BUILD_CACHE = {}


def _build(include_bd: bool, include_bu: bool, reps: int = 1):
    key = (include_bd, include_bu, reps)
    if key in _BUILD_CACHE:
        return _BUILD_CACHE[key]

    nc = bacc.Bacc("TRN2", target_bir_lowering=False, debug=False,
                   num_devices=N_CORES)
    xh_d = nc.dram_tensor("xh", [NLB * P, KC * LBLK], BF16,
                          kind="ExternalInput").ap()
    xl_d = nc.dram_tensor("xl", [NLB * P, KC * LBLK], BF16,
                          kind="ExternalInput").ap()
    wd_d = nc.dram_tensor("wd", [P, KC * EB], BF16, kind="ExternalInput").ap()
    wu_d = nc.dram_tensor("wu", [P, BC * D], BF16, kind="ExternalInput").ap()
    wgx_d = nc.dram_tensor("wgx", [P, KC, 2 * E], BF16,
                           kind="ExternalInput").ap()
    eb_d = nc.dram_tensor("eblk", [E, EB], BF16, kind="ExternalInput").ap()
    if include_bd:
        bd_d = nc.dram_tensor("bd", [P, BC], F32, kind="ExternalInput").ap()
    if include_bu:
        bu_d = nc.dram_tensor("bu", [E, D], BF16, kind="ExternalInput").ap()
    out_d = nc.dram_tensor("out", [TC, D], BF16, kind="ExternalOutput").ap()

    with tile.TileContext(nc) as tc, ExitStack() as ctx:
        wpool = ctx.enter_context(tc.tile_pool(name="weights", bufs=1))
        hgpool = ctx.enter_context(tc.tile_pool(name="hg", bufs=8))
        gpool = ctx.enter_context(tc.tile_pool(name="gates", bufs=2))
        opool = ctx.enter_context(tc.tile_pool(name="osb", bufs=3))

        ht_ps_pool = ctx.enter_context(
            tc.tile_pool(name="htps", bufs=3, space="PSUM"))
        lt_ps_pool = ctx.enter_context(
            tc.tile_pool(name="ltps", bufs=1, space="PSUM"))
        o_ps_pool = ctx.enter_context(
            tc.tile_pool(name="ops", bufs=4, space="PSUM"))

        if N_WARM:
            # tiny PE warm-up (p-state only; more burns HAM power budget)
            warm_bf = wpool.tile([P, EB], BF16, tag="warmbf")
            nc.vector.memset(warm_bf[:], 0.001)
            warm_ps = o_ps_pool.tile([P, EB], F32, tag="ops")
            for i in range(N_WARM):
                nc.tensor.matmul(warm_ps[:, 0:P], warm_bf[:, 0:P],
                                 warm_bf[:, 0:P], start=(i == 0),
                                 stop=(i == N_WARM - 1))

        # small constants first (fast), then the big streams in the order
        # the PE consumes them.
        wgx_sb = wpool.tile([P, KC, 2 * E], BF16, tag="wgx")
        nc.sync.dma_start(wgx_sb[:], wgx_d)
        eblk = wpool.tile([E, EB], BF16, tag="eblk")
        nc.sync.dma_start(eblk[:], eb_d)
        if include_bd:
            bd_sb = wpool.tile([P, BC], F32, tag="bd")
            nc.sync.dma_start(bd_sb[:], bd_d)
        if include_bu:
            bu_sb = wpool.tile([E, D], BF16, tag="bu")
            nc.sync.dma_start(bu_sb[:], bu_d)

        xh_sb = [wpool.tile([P, KC * LBLK], BF16, tag=f"xh{b}",
                            name=f"xh{b}") for b in range(NLB)]
        xl_sb = [wpool.tile([P, KC * LBLK], BF16, tag=f"xl{b}",
                            name=f"xl{b}") for b in range(NLB)]
        wd_sb = wpool.tile([P, KC * EB], BF16, tag="wd")
        wu_sb = wpool.tile([P, BC * D], BF16, tag="wu")

        HKC = KC // 2
        HW_COLS = HKC * LBLK  # half-slab columns

        def half(dst, src, brow, hh, eng=None):
            cols = bass.ts(hh, HW_COLS)
            (eng or nc.sync).dma_start(dst[:, cols],
                                       src[bass.ts(brow, P), cols])

        # stream order = PE consumption order
        half(xh_sb[0], xh_d, 0, 0)
        nc.sync.dma_start(wd_sb[:, 0:KC * EB // 2], wd_d[:, 0:KC * EB // 2])
        half(xh_sb[0], xh_d, 0, 1)
        nc.sync.dma_start(wd_sb[:, KC * EB // 2:], wd_d[:, KC * EB // 2:])
        half(xl_sb[0], xl_d, 0, 0)
        half(xl_sb[0], xl_d, 0, 1)
        half(xh_sb[1], xh_d, 1, 0)
        half(xh_sb[1], xh_d, 1, 1)
        half(xl_sb[1], xl_d, 1, 0)
        half(xl_sb[1], xl_d, 1, 1)
        nc.sync.dma_start(wu_sb[:], wu_d)

        def emit_logits_A(b, lt_ps, cs=None):
            for c in cs or range(KC):
                nc.tensor.matmul(lt_ps[:], wgx_sb[:, c, :],
                                 xh_sb[b][:, bass.ts(c, LBLK)],
                                 start=(c == 0), stop=False)

        def emit_logits_B(b, lt_ps):
            for c in range(KC):
                nc.tensor.matmul(lt_ps[:], wgx_sb[:, c, :],
                                 xl_sb[b][:, bass.ts(c, LBLK)],
                                 start=False, stop=(c == KC - 1))

        def emit_ht_pass(rep, b, ks, fillers=None):
            """Down-proj for EB chunks `ks`: c-outer so compute paces the
            wd DMA stream, one PSUM bank per chunk in the pass."""
            hts = [ht_ps_pool.tile([P, LBLK], F32, tag="htps",
                                   name=f"ht{rep}_{b}_{k}")
                   for k in ks]
            for c in range(KC):
                for i, k in enumerate(ks):
                    nc.tensor.matmul(
                        hts[i][:],
                        wd_sb[:, c * EB + k * P: c * EB + (k + 1) * P],
                        xh_sb[b][:, bass.ts(c, LBLK)],
                        start=(c == 0), stop=(c == KC - 1))
                if fillers is not None and c in fillers:
                    fillers[c]()
            rs = []
            for i, k in enumerate(ks):
                r_k = hgpool.tile([P, LBLK], BF16, tag="hg",
                                  name=f"hg{rep}_{b}_{k}")
                if include_bd:
                    nc.scalar.activation(r_k[:], hts[i][:], ACTF.Relu,
                                         bias=bd_sb[:, k:k + 1])
                else:
                    nc.scalar.activation(r_k[:], hts[i][:], ACTF.Relu)
                rs.append(r_k)
            return rs

        NB = LBLK // 32

        def emit_gating_block(rep, b, lt_ps):
            """Top-2 softmax (x0.5) for a whole 512-token block, entirely
            on DVE/ACT: 32x32 stream transpose puts each token's 16 logit
            halves contiguous in the free axis; the top-2 softmax is
            sigmoid(2l - m1 - m2) on the two masked lanes."""
            lt32 = gpool.tile([32, LBLK], F32, tag="lt32",
                              name=f"lt32_{rep}_{b}")
            nc.gpsimd.memset(lt32[:, :], 0.0)
            nc.scalar.copy(lt32[0:2 * E, :], lt_ps[:])
            ltT = gpool.tile([32, LBLK], F32, tag="ltT",
                             name=f"ltT_{rep}_{b}")
            nc.vector.transpose(ltT[:], lt32[:])
            v = ltT[:].rearrange("p (b c) -> p b c", c=32)
            l32 = gpool.tile([32, NB, E], F32, tag="l32")
            nc.vector.tensor_tensor(l32[:], v[:, :, 0:E], v[:, :, E:2 * E],
                                    op=AL.add)
            m1 = gpool.tile([32, NB], F32, tag="m1")
            nc.vector.tensor_reduce(m1[:], l32[:], AX.X, AL.max)
            m1b = m1[:].unsqueeze(2).broadcast_to([32, NB, E])
            mask1 = gpool.tile([32, NB, E], F32, tag="mask1")
            nc.vector.tensor_tensor(mask1[:], l32[:], m1b, op=AL.is_ge)
            lm = gpool.tile([32, NB, E], F32, tag="lm")
            nc.vector.scalar_tensor_tensor(lm[:], mask1[:], -1e30, l32[:],
                                           op0=AL.mult, op1=AL.add)
            m2 = gpool.tile([32, NB], F32, tag="m2")
            nc.vector.tensor_reduce(m2[:], lm[:], AX.X, AL.max)
            s = gpool.tile([32, NB], F32, tag="s")
            nc.vector.tensor_tensor(s[:], m1[:], m2[:], op=AL.add)
            sb = s[:].unsqueeze(2).broadcast_to([32, NB, E])
            arg = gpool.tile([32, NB, E], F32, tag="arg")
            nc.vector.scalar_tensor_tensor(arg[:], l32[:], 2.0, sb,
                                           op0=AL.mult, op1=AL.subtract)
            G = gpool.tile([32, NB, E], F32, tag="G")
            nc.scalar.activation(G[:], arg[:], ACTF.Sigmoid)
            m2b = m2[:].unsqueeze(2).broadcast_to([32, NB, E])
            mask2 = gpool.tile([32, NB, E], F32, tag="mask2")
            nc.vector.tensor_tensor(mask2[:], l32[:], m2b, op=AL.is_ge)
            g2blk = gpool.tile([32, LBLK], F32, tag="g2blk",
                               name=f"g2blk_{rep}_{b}")
            nc.gpsimd.memset(g2blk[:, :], 0.0)
            gv = g2blk[:].rearrange("p (b c) -> p b c", c=32)
            nc.vector.scalar_tensor_tensor(gv[:, :, 0:E], mask2[:], SCALE,
                                           G[:], op0=AL.mult, op1=AL.mult)
            g2tT = gpool.tile([32, LBLK], F32, tag="g2tT",
                              name=f"g2tT_{rep}_{b}")
            nc.vector.transpose(g2tT[:], g2blk[:])
            g2t_blk = gpool.tile([E, LBLK], BF16, tag="g2t",
                                 name=f"g2t_{rep}_{b}")
            nc.vector.tensor_copy(g2t_blk[:], g2tT[0:E, :])
            return g2t_blk

        def emit_hg(rep, b, rs, g2t_blk):
            """Expand per-token gates across each expert's 64 bottleneck
            lanes via the 0/1 block-matrix matmul, then scale the relu
            tiles (DVE)."""
            for k in range(BC):
                gb_ps = ht_ps_pool.tile([P, LBLK], F32, tag="htps",
                                        name=f"gb{rep}_{b}_{k}")
                nc.tensor.matmul(gb_ps[:], eblk[:, bass.ts(k, P)],
                                 g2t_blk[:], start=True, stop=True)
                nc.vector.tensor_tensor(rs[k][:], rs[k][:], gb_ps[:],
                                        op=AL.mult)

        def emit_out_tile(b, bo, rs, g2t_blk):
            t = b * TPB + bo
            o_sb = opool.tile([P, D], BF16, tag="osb")
            for h in range(2):
                o_ps = o_ps_pool.tile([P, 512], F32, tag="ops")
                n_b = BC + (1 if include_bu else 0)
                for k in range(BC):
                    nc.tensor.matmul(
                        o_ps[:], rs[k][:, bass.ts(bo, P)],
                        wu_sb[:, k * D + h * 512: k * D + (h + 1) * 512],
                        start=(k == 0), stop=(k == n_b - 1))
                if include_bu:
                    nc.tensor.matmul(o_ps[:],
                                     g2t_blk[:, bass.ts(bo, P)].bitcast(F32R),
                                     bu_sb[:, bass.ts(h, 512)],
                                     start=False, stop=True)
                if h == 0:
                    nc.vector.tensor_copy(o_sb[:, 0:512], o_ps[:])
                else:
                    nc.scalar.copy(o_sb[:, 512:1024], o_ps[:])
            nc.scalar.dma_start(out_d[bass.ts(t, P), :], o_sb[:])

        for rep in range(reps):
            # ---- block 0: logits + down-proj, paced by the DMA stream ----
            lt0 = lt_ps_pool.tile([2 * E, LBLK], F32, tag="ltps")
            emit_logits_A(0, lt0, cs=range(0, KC // 2))
            rs0 = emit_ht_pass(rep, 0, [0, 1], fillers={
                3: lambda: emit_logits_A(0, lt0, cs=range(KC // 2, KC))})
            rs0 += emit_ht_pass(rep, 0, [2, 3])
            emit_logits_B(0, lt0)
            g2t0 = emit_gating_block(rep, 0, lt0)   # DVE/ACT only
            lt1 = lt_ps_pool.tile([2 * E, LBLK], F32, tag="ltps")
            emit_logits_A(1, lt1)
            rs1 = emit_ht_pass(rep, 1, [0, 1])
            rs1 += emit_ht_pass(rep, 1, [2, 3])
            emit_logits_B(1, lt1)
            g2t1 = emit_gating_block(rep, 1, lt1)   # DVE/ACT only
            emit_hg(rep, 0, rs0, g2t0)
            for bo in range(TPB):
                emit_out_tile(0, bo, rs0, g2t0)
            emit_hg(rep, 1, rs1, g2t1)
            for bo in range(TPB):
                emit_out_tile(1, bo, rs1, g2t1)

    nc.compile()
    _BUILD_CACHE[key] = nc
    return nc


def _split_bf16(a):
    hi = a.astype(ml_dtypes.bfloat16)
    lo = (a - hi.astype(np.float32)).astype(ml_dtypes.bfloat16)
    return hi, lo


def _slab(a):
    """[TC, D] -> [NLB*P, KC*LBLK] block-major transposed layout."""
    return np.ascontiguousarray(
        a.reshape(NLB, LBLK, KC, P).transpose(0, 3, 2, 1)
        .reshape(NLB * P, KC * LBLK))


def kernel(x, w_gate, w_noise, Wd, bd, Wu, bu, reps: int = 1):
    x = np.ascontiguousarray(np.asarray(x, dtype=np.float32))
    assert x.shape == (B_DIM, S_DIM, D), x.shape
    wg = np.ascontiguousarray(np.asarray(w_gate, dtype=np.float32))
    Wd = np.asarray(Wd, dtype=np.float32)
    Wu = np.asarray(Wu, dtype=np.float32)
    bd = np.asarray(bd, dtype=np.float32)
    bu = np.asarray(bu, dtype=np.float32)

    include_bd = bool(np.any(bd))
    include_bu = bool(np.any(bu))
    nc = _build(include_bd, include_bu, reps)

    xf = x.reshape(T, D)
    xh, xl = _split_bf16(xf)
    wgh, wgl = _split_bf16(wg)
    # packed router weights [P, KC, 16]: [:, c, 0:8]=wgh chunk, [:, 8:16]=wgl
    wgx = np.ascontiguousarray(
        np.concatenate([wgh.reshape(KC, P, E), wgl.reshape(KC, P, E)],
                       axis=2).transpose(1, 0, 2))
    wd_all = Wd.transpose(1, 0, 2).reshape(D, EB).astype(ml_dtypes.bfloat16)
    wd_r = np.ascontiguousarray(
        wd_all.reshape(KC, P, EB).transpose(1, 0, 2).reshape(P, KC * EB))
    wu_flat = Wu.reshape(EB, D).astype(ml_dtypes.bfloat16)
    wu_r = np.ascontiguousarray(
        wu_flat.reshape(BC, P, D).transpose(1, 0, 2).reshape(P, BC * D))
    eblk = np.kron(np.eye(E), np.ones((1, BK))).astype(ml_dtypes.bfloat16)

    shared = dict(wd=wd_r, wu=wu_r, wgx=wgx, eblk=eblk)
    if include_bd:
        # [P, BC] partition-major per chunk: bd_sb[p, k] = bd_flat[128k+p]
        shared["bd"] = np.ascontiguousarray(
            bd.reshape(EB)[np.arange(P)[:, None] + P * np.arange(BC)[None]])
    if include_bu:
        shared["bu"] = np.ascontiguousarray(bu.astype(ml_dtypes.bfloat16))

    in_maps = []
    for c in range(N_CORES):
        sl = slice(c * TC, (c + 1) * TC)
        in_maps.append(dict(xh=_slab(xh[sl]), xl=_slab(xl[sl]), **shared))
    kernel.last_in_maps = in_maps
    res = run_bass_kernel_spmd(nc, in_maps, core_ids=list(range(N_CORES)))
    out = np.concatenate([np.asarray(res.results[c]["out"])
                          .astype(np.float32) for c in range(N_CORES)], axis=0)
    return out.reshape(B_DIM, S_DIM, D)


# revision 27
# speedup vs baseline: 1.0411x; 1.0411x over previous
"""MoE block (AdaptFormer adapters, top-2 of 8 experts) on 8 TRN2 NeuronCores.

Data-parallel over the 8192 tokens (1024/core), router + expert adapter
weights replicated. bf16 GEMMs, exact hi/lo router logits.

Per core:
  - x ships as an exact bf16 hi/lo split (xh + xl == x to ~2^-17),
    pre-transposed AND block-major on the host: [NLB*128, KC*512] where
    slab b holds block b's 512 tokens for all 8 D-chunks. Pure layout
    prep; lets block-0 compute start as soon as ~0.5MB has landed.
  - logits: one PSUM group of 16 matmuls per block computes
    (xh+xl) @ (wgh|wgl) with the packed [wgh|wgl] stationary: rows 0:8
    collect xh@wgh + xl@wgh, rows 8:16 collect xh@wgl + xl@wgl; the
    per-token transpose then a small add folds the halves. Exact to
    ~5e-6 (top-2/3 logit gaps below that are coin-flips worth <1e-2
    rel_l2 on this dataset).
  - gating runs entirely on DVE/ACT (no PE transposes): a 32x32 stream
    transpose lays each token's 16 logit halves contiguous in the free
    axis, the top-2 softmax is computed as sigmoid(2l - m1 - m2) on the
    two masked lanes (exactly softmax over the top 2), and a second
    stream transpose emits the [8, tok] gate matrix directly.
  - experts: HT = Wd^T-stationary bf16 matmuls on xh chunks (c-outer,
    paired PSUM banks so compute paces the wd stream) -> relu -> bf16;
    gates expand over the 512-wide expert axis via the 0/1 block-matrix
    matmul (GB); HG = relu * GB on DVE -> bf16; OUT tiles = HG-slices @
    Wu_flat (bf16) accumulated over the expert axis.
  - output is written bf16 [TC, D] and converted to f32 on host.
All experts computed densely; sparse gates zero the non-top-2 terms
(mathematically identical to dispatch/combine). The PE row budget is
kept minimal because TRN2's HAM power manager clamps sustained Tensor
throughput (~half rate until ~18us, then duty-cycled grants): warm-up
is 2 narrow matmuls, gating costs the PE nothing, and every GEMM is a
single bf16 pass.
"""
import numpy as np
import ml_dtypes
from contextlib import ExitStack

import concourse.bass as bass
import concourse.tile as tile
from concourse import bacc, mybir
from concourse.bass_utils import run_bass_kernel_spmd

N_CORES = 8
B_DIM, S_DIM, D = 2, 4096, 1024
T = B_DIM * S_DIM          # 8192 tokens
TC = T // N_CORES          # 1024 tokens per core
E, BK = 8, 64              # experts, bottleneck
EB = E * BK                # 512 concatenated expert axis
P = 128
KC = D // P                # D chunks
BC = EB // P               # bottleneck chunks
LBLK = 512                 # token block
NLB = TC // LBLK           # 2 blocks
TPB = LBLK // P            # token tiles per block
SCALE = 0.5
N_WARM = 2                 # PE warm-up matmuls: minimal (HAM power envelope -
                           # every wasted row costs grant budget)

F32 = mybir.dt.float32
F32R = mybir.dt.float32r
BF16 = mybir.dt.bfloat16
AL = mybir.AluOpType
ACTF = mybir.ActivationFunctionType
AX = mybir.AxisListType

_BUILD_CACHE = {}


def _build(include_bd: bool, include_bu: bool, reps: int = 1):
    key = (include_bd, include_bu, reps)
    if key in _BUILD_CACHE:
        return _BUILD_CACHE[key]

    nc = bacc.Bacc("TRN2", target_bir_lowering=False, debug=False,
                   num_devices=N_CORES)
    xh_d = nc.dram_tensor("xh", [NLB * P, KC * LBLK], BF16,
                          kind="ExternalInput").ap()
    xl_d = nc.dram_tensor("xl", [NLB * P, KC * LBLK], BF16,
                          kind="ExternalInput").ap()
    wd_d = nc.dram_tensor("wd", [P, KC * EB], BF16, kind="ExternalInput").ap()
    wu_d = nc.dram_tensor("wu", [P, BC * D], BF16, kind="ExternalInput").ap()
    wgx_d = nc.dram_tensor("wgx", [P, KC, 2 * E], BF16,
                           kind="ExternalInput").ap()
    eb_d = nc.dram_tensor("eblk", [E, EB], BF16, kind="ExternalInput").ap()
    if include_bd:
        bd_d = nc.dram_tensor("bd", [P, BC], F32, kind="ExternalInput").ap()
    if include_bu:
        bu_d = nc.dram_tensor("bu", [E, D], BF16, kind="ExternalInput").ap()
    out_d = nc.dram_tensor("out", [TC, D], BF16, kind="ExternalOutput").ap()

    with tile.TileContext(nc) as tc, ExitStack() as ctx:
        wpool = ctx.enter_context(tc.tile_pool(name="weights", bufs=1))
        hgpool = ctx.enter_context(tc.tile_pool(name="hg", bufs=8))
        gpool = ctx.enter_context(tc.tile_pool(name="gates", bufs=2))
        opool = ctx.enter_context(tc.tile_pool(name="osb", bufs=3))

        ht_ps_pool = ctx.enter_context(
            tc.tile_pool(name="htps", bufs=3, space="PSUM"))
        lt_ps_pool = ctx.enter_context(
            tc.tile_pool(name="ltps", bufs=1, space="PSUM"))
        o_ps_pool = ctx.enter_context(
            tc.tile_pool(name="ops", bufs=4, space="PSUM"))

        if N_WARM:
            # tiny PE warm-up (p-state only; more burns HAM power budget)
            warm_bf = wpool.tile([P, EB], BF16, tag="warmbf")
            nc.vector.memset(warm_bf[:], 0.001)
            warm_ps = o_ps_pool.tile([P, EB], F32, tag="ops")
            for i in range(N_WARM):
                nc.tensor.matmul(warm_ps[:, 0:P], warm_bf[:, 0:P],
                                 warm_bf[:, 0:P], start=(i == 0),
                                 stop=(i == N_WARM - 1))

        # small constants first (fast), then the big streams in the order
        # the PE consumes them.
        wgx_sb = wpool.tile([P, KC, 2 * E], BF16, tag="wgx")
        nc.sync.dma_start(wgx_sb[:], wgx_d)
        eblk = wpool.tile([E, EB], BF16, tag="eblk")
        nc.sync.dma_start(eblk[:], eb_d)
        if include_bd:
            bd_sb = wpool.tile([P, BC], F32, tag="bd")
            nc.sync.dma_start(bd_sb[:], bd_d)
        if include_bu:
            bu_sb = wpool.tile([E, D], BF16, tag="bu")
            nc.sync.dma_start(bu_sb[:], bu_d)

        xh_sb = [wpool.tile([P, KC * LBLK], BF16, tag=f"xh{b}",
                            name=f"xh{b}") for b in range(NLB)]
        xl_sb = [wpool.tile([P, KC * LBLK], BF16, tag=f"xl{b}",
                            name=f"xl{b}") for b in range(NLB)]
        wd_sb = wpool.tile([P, KC * EB], BF16, tag="wd")
        wu_sb = wpool.tile([P, BC * D], BF16, tag="wu")

        HKC = KC // 2
        HW_COLS = HKC * LBLK  # half-slab columns

        def half(dst, src, brow, hh, eng=None):
            cols = bass.ts(hh, HW_COLS)
            (eng or nc.sync).dma_start(dst[:, cols],
                                       src[bass.ts(brow, P), cols])

        # stream order = PE consumption order
        half(xh_sb[0], xh_d, 0, 0)
        nc.sync.dma_start(wd_sb[:, 0:KC * EB // 2], wd_d[:, 0:KC * EB // 2])
        half(xh_sb[0], xh_d, 0, 1)
        nc.sync.dma_start(wd_sb[:, KC * EB // 2:], wd_d[:, KC * EB // 2:])
        half(xl_sb[0], xl_d, 0, 0)
        half(xl_sb[0], xl_d, 0, 1)
        half(xh_sb[1], xh_d, 1, 0)
        half(xh_sb[1], xh_d, 1, 1)
        half(xl_sb[1], xl_d, 1, 0)
        half(xl_sb[1], xl_d, 1, 1)
        nc.sync.dma_start(wu_sb[:], wu_d)

        def emit_logits_A(b, lt_ps, cs=None):
            for c in cs or range(KC):
                nc.tensor.matmul(lt_ps[:], wgx_sb[:, c, :],
                                 xh_sb[b][:, bass.ts(c, LBLK)],
                                 start=(c == 0), stop=False)

        def emit_logits_B(b, lt_ps):
            for c in range(KC):
                nc.tensor.matmul(lt_ps[:], wgx_sb[:, c, :],
                                 xl_sb[b][:, bass.ts(c, LBLK)],
                                 start=False, stop=(c == KC - 1))

        def emit_ht_pass(rep, b, ks, fillers=None):
            """Down-proj for EB chunks `ks`: c-outer so compute paces the
            wd DMA stream, one PSUM bank per chunk in the pass."""
            hts = [ht_ps_pool.tile([P, LBLK], F32, tag="htps",
                                   name=f"ht{rep}_{b}_{k}")
                   for k in ks]
            for c in range(KC):
                for i, k in enumerate(ks):
                    nc.tensor.matmul(
                        hts[i][:],
                        wd_sb[:, c * EB + k * P: c * EB + (k + 1) * P],
                        xh_sb[b][:, bass.ts(c, LBLK)],
                        start=(c == 0), stop=(c == KC - 1))
                if fillers is not None and c in fillers:
                    fillers[c]()
            rs = []
            for i, k in enumerate(ks):
                r_k = hgpool.tile([P, LBLK], BF16, tag="hg",
                                  name=f"hg{rep}_{b}_{k}")
                if include_bd:
                    nc.scalar.activation(r_k[:], hts[i][:], ACTF.Relu,
                                         bias=bd_sb[:, k:k + 1])
                else:
                    nc.scalar.activation(r_k[:], hts[i][:], ACTF.Relu)
                rs.append(r_k)
            return rs

        NB = LBLK // 32

        def emit_gating_block(rep, b, lt_ps):
            """Top-2 softmax (x0.5) for a whole 512-token block, entirely
            on DVE/ACT: 32x32 stream transpose puts each token's 16 logit
            halves contiguous in the free axis; the top-2 softmax is
            sigmoid(2l - m1 - m2) on the two masked lanes."""
            lt32 = gpool.tile([32, LBLK], F32, tag="lt32",
                              name=f"lt32_{rep}_{b}")
            nc.gpsimd.memset(lt32[:, :], 0.0)
            nc.scalar.copy(lt32[0:2 * E, :], lt_ps[:])
            ltT = gpool.tile([32, LBLK], F32, tag="ltT",
                             name=f"ltT_{rep}_{b}")
            nc.vector.transpose(ltT[:], lt32[:])
            v = ltT[:].rearrange("p (b c) -> p b c", c=32)
            l32 = gpool.tile([32, NB, E], F32, tag="l32")
            nc.vector.tensor_tensor(l32[:], v[:, :, 0:E], v[:, :, E:2 * E],
                                    op=AL.add)
            m1 = gpool.tile([32, NB], F32, tag="m1")
            nc.vector.tensor_reduce(m1[:], l32[:], AX.X, AL.max)
            m1b = m1[:].unsqueeze(2).broadcast_to([32, NB, E])
            mask1 = gpool.tile([32, NB, E], F32, tag="mask1")
            nc.vector.tensor_tensor(mask1[:], l32[:], m1b, op=AL.is_ge)
            lm = gpool.tile([32, NB, E], F32, tag="lm")
            nc.vector.scalar_tensor_tensor(lm[:], mask1[:], -1e30, l32[:],
                                           op0=AL.mult, op1=AL.add)
            m2 = gpool.tile([32, NB], F32, tag="m2")
            nc.vector.tensor_reduce(m2[:], lm[:], AX.X, AL.max)
            s = gpool.tile([32, NB], F32, tag="s")
            nc.vector.tensor_tensor(s[:], m1[:], m2[:], op=AL.add)
            sb = s[:].unsqueeze(2).broadcast_to([32, NB, E])
            arg = gpool.tile([32, NB, E], F32, tag="arg")
            nc.vector.scalar_tensor_tensor(arg[:], l32[:], 2.0, sb,
                                           op0=AL.mult, op1=AL.subtract)
            G = gpool.tile([32, NB, E], F32, tag="G")
            nc.scalar.activation(G[:], arg[:], ACTF.Sigmoid)
            m2b = m2[:].unsqueeze(2).broadcast_to([32, NB, E])
            mask2 = gpool.tile([32, NB, E], F32, tag="mask2")
            nc.vector.tensor_tensor(mask2[:], l32[:], m2b, op=AL.is_ge)
            g2blk = gpool.tile([32, LBLK], F32, tag="g2blk",
                               name=f"g2blk_{rep}_{b}")
            nc.gpsimd.memset(g2blk[:, :], 0.0)
            gv = g2blk[:].rearrange("p (b c) -> p b c", c=32)
            nc.vector.scalar_tensor_tensor(gv[:, :, 0:E], mask2[:], SCALE,
                                           G[:], op0=AL.mult, op1=AL.mult)
            g2tT = gpool.tile([32, LBLK], F32, tag="g2tT",
                              name=f"g2tT_{rep}_{b}")
            nc.vector.transpose(g2tT[:], g2blk[:])
            g2t_blk = gpool.tile([E, LBLK], BF16, tag="g2t",
                                 name=f"g2t_{rep}_{b}")
            nc.vector.tensor_copy(g2t_blk[:], g2tT[0:E, :])
            return g2t_blk

        def emit_hg(rep, b, rs, g2t_blk):
            """Expand per-token gates across each expert's 64 bottleneck
            lanes via the 0/1 block-matrix matmul, then scale the relu
            tiles (DVE)."""
            for k in range(BC):
                gb_ps = ht_ps_pool.tile([P, LBLK], F32, tag="htps",
                                        name=f"gb{rep}_{b}_{k}")
                nc.tensor.matmul(gb_ps[:], eblk[:, bass.ts(k, P)],
                                 g2t_blk[:], start=True, stop=True)
                nc.vector.tensor_tensor(rs[k][:], rs[k][:], gb_ps[:],
                                        op=AL.mult)

        def emit_out_tile(b, bo, rs, g2t_blk):
            t = b * TPB + bo
            o_sb = opool.tile([P, D], BF16, tag="osb")
            for h in range(2):
                o_ps = o_ps_pool.tile([P, 512], F32, tag="ops")
                n_b = BC + (1 if include_bu else 0)
                for k in range(BC):
                    nc.tensor.matmul(
                        o_ps[:], rs[k][:, bass.ts(bo, P)],
                        wu_sb[:, k * D + h * 512: k * D + (h + 1) * 512],
                        start=(k == 0), stop=(k == n_b - 1))
                if include_bu:
                    nc.tensor.matmul(o_ps[:],
                                     g2t_blk[:, bass.ts(bo, P)].bitcast(F32R),
                                     bu_sb[:, bass.ts(h, 512)],
                                     start=False, stop=True)
                if h == 0:
                    nc.vector.tensor_copy(o_sb[:, 0:512], o_ps[:])
                else:
                    nc.scalar.copy(o_sb[:, 512:1024], o_ps[:])
            nc.scalar.dma_start(out_d[bass.ts(t, P), :], o_sb[:])

        for rep in range(reps):
            # ---- block 0: logits + down-proj, paced by the DMA stream ----
            lt0 = lt_ps_pool.tile([2 * E, LBLK], F32, tag="ltps")
            emit_logits_A(0, lt0, cs=range(0, KC // 2))
            rs0 = emit_ht_pass(rep, 0, [0, 1], fillers={
                3: lambda: emit_logits_A(0, lt0, cs=range(KC // 2, KC))})
            rs0 += emit_ht_pass(rep, 0, [2, 3])
            emit_logits_B(0, lt0)
            g2t0 = emit_gating_block(rep, 0, lt0)   # DVE/ACT only
            lt1 = lt_ps_pool.tile([2 * E, LBLK], F32, tag="ltps")
            emit_logits_A(1, lt1)
            rs1 = emit_ht_pass(rep, 1, [0, 1])
            rs1 += emit_ht_pass(rep, 1, [2, 3])
            emit_logits_B(1, lt1)
            g2t1 = emit_gating_block(rep, 1, lt1)   # DVE/ACT only
            emit_hg(rep, 0, rs0, g2t0)
            for bo in range(TPB):
                emit_out_tile(0, bo, rs0, g2t0)
            emit_hg(rep, 1, rs1, g2t1)
            for bo in range(TPB):
                emit_out_tile(1, bo, rs1, g2t1)

    nc.compile()
    _BUILD_CACHE[key] = nc
    return nc


def _split_bf16(a):
    hi = a.astype(ml_dtypes.bfloat16)
    lo = (a - hi.astype(np.float32)).astype(ml_dtypes.bfloat16)
    return hi, lo


def _slab(a):
    """[TC, D] -> [NLB*P, KC*LBLK] block-major transposed layout."""
    return np.ascontiguousarray(
        a.reshape(NLB, LBLK, KC, P).transpose(0, 3, 2, 1)
        .reshape(NLB * P, KC * LBLK))


def kernel(x, w_gate, w_noise, Wd, bd, Wu, bu, reps: int = 1):
    x = np.ascontiguousarray(np.asarray(x, dtype=np.float32))
    assert x.shape == (B_DIM, S_DIM, D), x.shape
    wg = np.ascontiguousarray(np.asarray(w_gate, dtype=np.float32))
    Wd = np.asarray(Wd, dtype=np.float32)
    Wu = np.asarray(Wu, dtype=np.float32)
    bd = np.asarray(bd, dtype=np.float32)
    bu = np.asarray(bu, dtype=np.float32)

    include_bd = bool(np.any(bd))
    include_bu = bool(np.any(bu))
    nc = _build(include_bd, include_bu, reps)

    xf = x.reshape(T, D)
    xh, xl = _split_bf16(xf)
    wgh, wgl = _split_bf16(wg)
    # packed router weights [P, KC, 16]: [:, c, 0:8]=wgh chunk, [:, 8:16]=wgl
    wgx = np.ascontiguousarray(
        np.concatenate([wgh.reshape(KC, P, E), wgl.reshape(KC, P, E)],
                       axis=2).transpose(1, 0, 2))
    wd_all = Wd.transpose(1, 0, 2).reshape(D, EB).astype(ml_dtypes.bfloat16)
    wd_r = np.ascontiguousarray(
        wd_all.reshape(KC, P, EB).transpose(1, 0, 2).reshape(P, KC * EB))
    wu_flat = Wu.reshape(EB, D).astype(ml_dtypes.bfloat16)
    wu_r = np.ascontiguousarray(
        wu_flat.reshape(BC, P, D).transpose(1, 0, 2).reshape(P, BC * D))
    eblk = np.kron(np.eye(E), np.ones((1, BK))).astype(ml_dtypes.bfloat16)

    shared = dict(wd=wd_r, wu=wu_r, wgx=wgx, eblk=eblk)
    if include_bd:
        # [P, BC] partition-major per chunk: bd_sb[p, k] = bd_flat[128k+p]
        shared["bd"] = np.ascontiguousarray(
            bd.reshape(EB)[np.arange(P)[:, None] + P * np.arange(BC)[None]])
    if include_bu:
        shared["bu"] = np.ascontiguousarray(bu.astype(ml_dtypes.bfloat16))

    in_maps = []
    for c in range(N_CORES):
        sl = slice(c * TC, (c + 1) * TC)
        in_maps.append(dict(xh=_slab(xh[sl]), xl=_slab(xl[sl]), **shared))
    kernel.last_in_maps = in_maps
    res = run_bass_kernel_spmd(nc, in_maps, core_ids=list(range(N_CORES)))
    out = np.concatenate([np.asarray(res.results[c]["out"])
                          .astype(np.float32) for c in range(N_CORES)], axis=0)
    return out.reshape(B_DIM, S_DIM, D)
